# revision 54
# baseline (speedup 1.0000x reference)
"""Causal multi-head attention (B=2, T=2048, DIM=1024, H=16) on 8 TRN2 cores.

Sharding: core c handles batch b = c // 4 and head-group g = c % 4 (4 heads,
head-dim slice of 256).  Each core computes QKV projections for its heads,
causal attention, and a partial output projection y_partial of shape
(2048, 1024).  Host sums the 4 partials per batch (the tensor-parallel
all-reduce, done as the unshard step).

All matmuls run in bf16 (full PE rate, no fp32r narrow-moving penalty);
PSUM accumulation is f32.  End-to-end absmax rel-err vs the fp32 reference
is ~4e-3 (gate 2e-2).

Schedule: one fused PE instruction stream produced by a build-time
cost-tracking emitter.  Attention steps (scores -> exp on ACT -> attn@v)
are interleaved with "filler" matmuls (QKV projections of later
token-quarters and of the NEXT iteration, plus output projections of
earlier query groups) so the PE never waits on the ACT engine's exp.
Weights/x/q/k/v/o buffers are double-buffered per iteration, letting
consecutive timing-loop iterations pipeline into each other.  Softmax
denominators ride along as a ones-column of v; normalization happens off
the critical path (DVE copy + reciprocal + DMA-broadcast + deferred Pool
multiply), one query-group behind the attention wavefront.
"""

import os
import sys

sys.path.insert(0, "/opt/trn_rl_repo")

from collections import deque

import numpy as np

DEBUG_EMITS = []   # (instruction_name, description) when KDBG=1

B, T, DIM, H = 2, 2048, 1024, 16
HD = DIM // H          # 64
NCORES = 8
GROUPS = 4             # head-groups (4 heads each)
GH = H // GROUPS       # 4 heads per group
DH = GH * HD           # 256 head dims per group
NPAIR = 2              # pairs of heads (2 heads = 128 partitions)
TT = T // 128          # 16 token tiles
TG = T // 512          # 4 query groups of 512
KO = DIM // 128        # 8 contraction chunks

# emitter cost model (ns)
PE_C = 1.0 / 2.4       # PE cycle at full pstate
ACT_C = 1.0 / 1.2      # ACT cycle
EXP_OVH = 185.0        # per-exp fixed overhead (access latency)
SEM = 220.0            # semaphore propagation margin


def _build_program(loop=1):
    import concourse.bass as bass
    import concourse.tile as tile
    from concourse import bacc, mybir

    F32 = mybir.dt.float32
    BF16 = mybir.dt.bfloat16
    AF = mybir.ActivationFunctionType

    nc = bacc.Bacc("TRN2", target_bir_lowering=False, debug=False,
                   num_devices=NCORES)

    if os.environ.get("KDBG"):
        DEBUG_EMITS.clear()
        _orig_mm = nc.tensor.matmul

        def _mm(*a, **k):
            inst = _orig_mm(*a, **k)
            DEBUG_EMITS.append((inst.ins.name, _mm.desc))
            return inst
        _mm.desc = "init"
        nc.tensor.matmul = _mm

        def _set_desc(d):
            _mm.desc = d
    else:
        def _set_desc(d):
            pass

    xt_d = nc.dram_tensor("xt", [DIM, T], BF16, kind="ExternalInput")
    wqt_d = nc.dram_tensor("wqt", [DIM, DH], BF16, kind="ExternalInput")
    wkt_d = nc.dram_tensor("wkt", [DIM, DH], BF16, kind="ExternalInput")
    wvt_d = nc.dram_tensor("wvt", [DIM, DH], BF16, kind="ExternalInput")
    wot_d = nc.dram_tensor("wot", [DH, DIM], BF16, kind="ExternalInput")
    y_d = nc.dram_tensor("y", [T, DIM], F32, kind="ExternalOutput")

    with tile.TileContext(nc) as tc:
        with (
            tc.tile_pool(name="singles", bufs=1) as singles,
            tc.tile_pool(name="pabp", bufs=5) as pabp,
            tc.tile_pool(name="worky", bufs=6) as worky,
            tc.tile_pool(name="tiny", bufs=3) as tiny,
            tc.tile_pool(name="sp", bufs=2, space="PSUM") as sp,
            tc.tile_pool(name="opool", bufs=1, space="PSUM") as opool,
            tc.tile_pool(name="accp", bufs=2, space="PSUM") as accp,
            tc.tile_pool(name="dramp", bufs=2, space="DRAM") as dramp,
        ):
            # ---- persistent SBUF tensors ----
            maskf = singles.tile([128, 128], F32)
            nc.gpsimd.memset(maskf[:], 1.0)
            # keep 1 where q - k >= 0 (k on partitions, q on free), else 0
            nc.gpsimd.affine_select(
                out=maskf[:], in_=maskf[:],
                compare_op=mybir.AluOpType.is_ge, fill=0.0,
                base=0, pattern=[[1, 128]], channel_multiplier=-1,
            )
            mask01 = singles.tile([128, 128], BF16)
            nc.vector.tensor_copy(mask01[:], maskf[:])

            ones_f = singles.tile([128, HD], F32)
            nc.vector.memset(ones_f[:], 1.0)
            onesb = singles.tile([1, HD], BF16)
            nc.vector.tensor_copy(onesb[:], ones_f[0:1, :])
            # warm the ACT exp table during the initial DMA
            dummy = singles.tile([128, 1], F32)
            nc.scalar.activation(dummy[:], ones_f[:, 0:1], AF.Exp)

            wpool = tc.alloc_tile_pool(name="wpool", bufs=2)
            xqp = tc.alloc_tile_pool(name="xqp", bufs=2)
            # double-buffered per-iteration q/k/v/o tensors so iteration
            # it+1's projections can overlap iteration it's attention
            qkvp = tc.alloc_tile_pool(name="qkvp", bufs=2)
            xt_r = xt_d.rearrange("(ko p) t -> p ko t", p=128)
            wq_r = wqt_d.rearrange("(ko p) d -> p ko d", p=128)
            wk_r = wkt_d.rearrange("(ko p) d -> p ko d", p=128)
            wv_r = wvt_d.rearrange("(ko p) d -> p ko d", p=128)
            wo_r = wot_d.rearrange("(ko p) j -> p ko j", p=128)

            xq = {}    # (it, Q) -> x tile

            wsb = {}   # it -> weight tiles

            def load_quarter(it, Q):
                t_ = xqp.tile([128, KO, 512], BF16, tag="xq",
                              name=f"xq{it}_{Q}")
                xq[(it, Q)] = t_
                nc.sync.dma_start(t_, xt_r[:, :, 512 * Q:512 * (Q + 1)])

            def initial_loads(it):
                w = {
                    "q": wpool.tile([128, KO, DH], BF16, tag="wq",
                                    name=f"wq{it}"),
                    "k": wpool.tile([128, KO, DH], BF16, tag="wk",
                                    name=f"wk{it}"),
                    "v": wpool.tile([128, KO, DH], BF16, tag="wv",
                                    name=f"wv{it}"),
                    "o": wpool.tile([128, DH // 128, DIM], BF16, tag="wo",
                                    name=f"wo{it}"),
                    "qT": qkvp.tile([128, NPAIR, T], BF16, tag="qT",
                                    name=f"qT{it}"),
                    "kT": qkvp.tile([128, NPAIR, T], BF16, tag="kT",
                                    name=f"kT{it}"),
                    "oT": qkvp.tile([128, NPAIR, T], BF16, tag="oT",
                                    name=f"oT{it}"),
                    "vt": qkvp.tile([128, TT, GH, HD + 1], BF16, tag="vt",
                                    name=f"vt{it}"),
                }
                wsb[it] = w
                # ones column of v (softmax denominator accumulator)
                for h in range(GH):
                    nc.vector.tensor_copy(w["vt"][:, :, h, HD:HD + 1],
                                          ones_f[:, 0:TT, None])
                if it == 0:
                    # fine-grained first loads: the first matmuls can start
                    # once the first ko-chunks of wq and xq0 arrive
                    t0 = xqp.tile([128, KO, 512], BF16, tag="xq",
                                  name=f"xq{it}_0")
                    xq[(it, 0)] = t0
                    for h in range(4):
                        nc.sync.dma_start(w["q"][:, 2 * h:2 * (h + 1), :],
                                          wq_r[:, 2 * h:2 * (h + 1), :])
                        nc.sync.dma_start(t0[:, 2 * h:2 * (h + 1), :],
                                          xt_r[:, 2 * h:2 * (h + 1), 0:512])
                    nc.sync.dma_start(w["k"], wk_r)
                    load_quarter(it, 1)
                    nc.sync.dma_start(w["v"], wv_r)
                    nc.sync.dma_start(w["o"], wo_r)
                else:
                    nc.sync.dma_start(w["q"], wq_r)
                    load_quarter(it, 0)
                    nc.sync.dma_start(w["k"], wk_r)
                    load_quarter(it, 1)
                    nc.sync.dma_start(w["v"], wv_r)
                    nc.sync.dma_start(w["o"], wo_r)

            # ---------- filler stream (PE-only work) ----------
            filler = []      # (fn, pe_ns)
            marks = {}       # dep key -> filler index that must be emitted

            def qk_units(it, Q, which, p):
                box = {}
                for k2 in range(4):
                    def fn(k2=k2, it=it, Q=Q, p=p, box=box, which=which):
                        _set_desc(f"proj-{which}{it}.Q{Q}p{p}k{k2}")
                        w_sb = wsb[it]["q" if which == "q" else "k"]
                        dstT = wsb[it]["qT" if which == "q" else "kT"]
                        if k2 == 0:
                            box["t"] = accp.tile(
                                [128, 512], F32, tag="a",
                                name=f"{which}{it}_{Q}_{p}")
                        acc = box["t"]
                        for ko in (2 * k2, 2 * k2 + 1):
                            nc.tensor.matmul(
                                acc[:], w_sb[:, ko, 128 * p:128 * (p + 1)],
                                xq[(it, Q)][:, ko, :],
                                start=(ko == 0), stop=(ko == KO - 1))
                        if k2 == 3:
                            nc.vector.tensor_copy(
                                dstT[:, p, 512 * Q:512 * (Q + 1)], acc[:])
                    filler.append((fn, 2 * 512 * PE_C))
                marks[(it, which, Q, p)] = len(filler)

            def v_units(it, Q):
                for tt in range(4 * Q, 4 * Q + 4):
                    box = {}
                    for h2 in range(2):
                        def fn(tt=tt, h2=h2, it=it, Q=Q, box=box):
                            _set_desc(f"proj-v{it}.t{tt}h{h2}")
                            if h2 == 0:
                                box["t"] = accp.tile(
                                    [128, 512], F32, tag="a",
                                    name=f"v{it}_{tt}")
                            acc = box["t"]
                            for ko in range(4 * h2, 4 * h2 + 4):
                                nc.tensor.matmul(
                                    acc[:, 0:DH],
                                    xq[(it, Q)][:, ko,
                                                128 * (tt % 4):128 * (tt % 4 + 1)],
                                    wsb[it]["v"][:, ko, :],
                                    start=(ko == 0), stop=(ko == KO - 1))
                            if h2 == 1:
                                nc.vector.tensor_copy(
                                    wsb[it]["vt"][:, tt, :, 0:HD],
                                    acc[:, 0:DH].rearrange(
                                        "p (h d) -> p h d", h=GH))
                        filler.append((fn, 4 * DH * PE_C))
                    marks[(it, "v", tt)] = len(filler)

            for it in range(loop):
                filler.append((lambda it=it: initial_loads(it), 0.0))
                for Q in range(4):
                    qk_units(it, Q, "q", 0)
                    qk_units(it, Q, "q", 1)
                    qk_units(it, Q, "k", 0)
                    qk_units(it, Q, "k", 1)
                    v_units(it, Q)
                    if Q < 2:
                        filler.append(
                            (lambda it=it, Q=Q + 2: load_quarter(it, Q), 0.0))

            # ---------- attention steps ----------
            steps = []
            for it in range(loop):
                for G in range(TG):
                    for p in range(NPAIR):
                        for j in range(4 * G + 4):
                            steps.append((it, G, p, j))
            N = len(steps)
            ptile = [None] * N
            expEnd = [0.0] * N
            st = {"peT": 0.0, "actFree": 0.0, "fi": 0, "o": None,
                  "oFree": 0.0}
            yq = deque()   # (ready_at_peT, fn, pe_ns)

            if True:

                def emit_filler_one():
                    fn, c = filler[st["fi"]]
                    st["fi"] += 1
                    fn()
                    st["peT"] += c

                def need(pos):
                    while st["fi"] < pos:
                        emit_filler_one()

                def scores_dep(i):
                    it, G, p, j = steps[i]
                    return max(marks[(it, "q", G, p)],
                               marks[(it, "k", j // 4, p)])

                def emit_scores(i):
                    it, G, p, j = steps[i]
                    d = j - 4 * G
                    off = max(0, d) * 128
                    need(scores_dep(i))
                    _set_desc(f"scores{it}.G{G}p{p}j{j}")
                    s = sp.tile([128, 1024], F32, tag="s", name=f"s_{i}")
                    qs = slice(512 * G + off, 512 * (G + 1))
                    ks = slice(128 * j, 128 * (j + 1))
                    qTt, kTt = wsb[it]["qT"], wsb[it]["kT"]
                    nc.tensor.matmul(s[:, off:512], kTt[0:64, p, ks],
                                     qTt[0:64, p, qs], start=True, stop=True)
                    nc.tensor.matmul(s[:, 512:1024 - off], kTt[64:128, p, ks],
                                     qTt[64:128, p, qs], start=True, stop=True)
                    st["peT"] += 2 * (512 - off) * PE_C
                    pab = pabp.tile([128, 1024], BF16, tag="pab",
                                    name=f"pab_{i}")
                    nc.scalar.activation(pab[:, off:1024 - off],
                                         s[:, off:1024 - off], AF.Exp)
                    e = max(st["peT"] + SEM, st["actFree"]) \
                        + (1024 - 2 * off) * ACT_C + EXP_OVH
                    st["actFree"] = e
                    if d >= 0:
                        a = pab[:, off:off + 128]
                        dst = bass.AP(tensor=a.tensor, offset=a.offset,
                                      ap=[list(a.ap)[0], [512 - off, 2],
                                          list(a.ap)[-1]])
                        nc.vector.tensor_mul(
                            dst, dst,
                            mask01[:, None, :].to_broadcast((128, 2, 128)))
                        e += SEM + 250.0 + SEM
                    expEnd[i] = e
                    ptile[i] = pab

                def norm_chain(it, G, p):
                    o = st["o"]
                    oTt = wsb[it]["oT"]
                    qsl = slice(512 * G, 512 * (G + 1))
                    # o PSUM banks stay busy until the oU staging copy lands
                    st["oFree"] = st["peT"] + 1600.0
                    last = (it == loop - 1 and G == TG - 1 and p == NPAIR - 1)
                    if last:
                        # tail fast-path: no DMA round-trip — broadcast
                        # 1/denom to 64 partitions via PE rank-1 matmuls
                        # (2x512: moving free dim is capped at 512); stage o
                        # to SBUF meanwhile (TensorTensor allows only one
                        # PSUM input)
                        r0 = tiny.tile([1, 1024], BF16, tag="r0b",
                                       name=f"r0b{it}")
                        with nc.allow_low_precision(
                                reason="bf16 1/denom for rank-1 broadcast"):
                            nc.vector.reciprocal(r0[:], o[HD:HD + 1, :])
                        rb = sp.tile([128, 1024], F32, tag="s",
                                     name=f"rb{it}")
                        nc.tensor.matmul(rb[0:HD, 0:512], onesb[:],
                                         r0[:, 0:512], start=True, stop=True)
                        nc.tensor.matmul(rb[0:HD, 512:1024], onesb[:],
                                         r0[:, 512:1024], start=True,
                                         stop=True)
                        st["peT"] += 1024 * PE_C
                        # copy o into place, then scale in-place (each op
                        # reads at most one PSUM operand)
                        nc.vector.tensor_copy(oTt[0:64, p, qsl],
                                              o[0:HD, 0:512])
                        nc.vector.tensor_copy(oTt[64:128, p, qsl],
                                              o[0:HD, 512:1024])
                        nc.vector.tensor_mul(oTt[0:64, p, qsl],
                                             oTt[0:64, p, qsl],
                                             rb[0:HD, 0:512])
                        nc.vector.tensor_mul(oTt[64:128, p, qsl],
                                             oTt[64:128, p, qsl],
                                             rb[0:HD, 512:1024])
                    else:
                        oU = tiny.tile([HD + 1, 1024], F32, tag="oU",
                                       name=f"oU{it}_{G}_{p}")
                        nc.vector.tensor_copy(oU[:], o[:])
                        r0 = tiny.tile([1, 1024], F32, tag="r0",
                                       name=f"r0_{it}_{G}_{p}")
                        nc.vector.reciprocal(r0[:], oU[HD:HD + 1, :])
                        rdr = dramp.tile([1, 1024], F32,
                                         name=f"rdr{it}_{G}_{p}")
                        nc.sync.dma_start(rdr[:], r0[:])
                        Rsb = tiny.tile([HD, 1024], F32, tag="Rsb",
                                        name=f"Rsb{it}_{G}_{p}")
                        rap = rdr[:]
                        bc = bass.AP(tensor=rap.tensor, offset=rap.offset,
                                     ap=[[0, HD]] + list(rap.ap)[1:])
                        nc.sync.dma_start(Rsb[:], bc)

                        # the broadcast takes a DMA round-trip; defer the
                        # multiplies so they don't head-of-line-block Pool
                        def normfn(oTt=oTt, p=p, oU=oU, Rsb=Rsb, qsl=qsl):
                            nc.gpsimd.tensor_mul(oTt[0:64, p, qsl],
                                                 oU[0:HD, 0:512],
                                                 Rsb[:, 0:512])
                            nc.gpsimd.tensor_mul(oTt[64:128, p, qsl],
                                                 oU[0:HD, 512:1024],
                                                 Rsb[:, 512:1024])
                        yq.append((st["peT"] + 3500.0, normfn, 0.0))
                    if p == NPAIR - 1:
                        ready = st["peT"] + (0.0 if last else 5000.0)
                        for tt in range(4 * G, 4 * G + 4):
                            for jh in range(2):
                                def yfn(tt=tt, jh=jh, G=G, it=it):
                                    _set_desc(f"y{it}.G{G}t{tt}h{jh}")
                                    acc = accp.tile([128, 512], F32, tag="a",
                                                    name=f"y{it}_{tt}_{jh}")
                                    for p2 in range(NPAIR):
                                        nc.tensor.matmul(
                                            acc[:],
                                            wsb[it]["oT"][:, p2,
                                                          128 * tt:128 * (tt + 1)],
                                            wsb[it]["o"][:, p2,
                                                         512 * jh:512 * (jh + 1)],
                                            start=(p2 == 0),
                                            stop=(p2 == NPAIR - 1))
                                    ysb = worky.tile([128, 512], F32, tag="y",
                                                     name=f"ysb{it}_{tt}_{jh}")
                                    # G2's y drains pop while ACT is still
                                    # exp-saturated (next iter's S3) — keep
                                    # them off ACT so the acc tile frees fast
                                    if jh == 0 or G == 2:
                                        nc.vector.tensor_copy(ysb[:], acc[:])
                                    else:
                                        nc.scalar.copy(ysb[:], acc[:])
                                    nc.sync.dma_start(
                                        y_d[128 * tt:128 * (tt + 1),
                                            512 * jh:512 * (jh + 1)], ysb[:])
                                yq.append((ready, yfn, 2 * 512 * PE_C))

                def emit_attn(i):
                    it, G, p, j = steps[i]
                    d = j - 4 * G
                    off = max(0, d) * 128
                    njt = 4 * G + 4
                    need(marks[(it, "v", j)])
                    if j == 0:
                        st["o"] = opool.tile([HD + 1, 1024], F32, tag="o",
                                             name=f"o{it}_{G}_{p}")
                    _set_desc(f"attn{it}.G{G}p{p}j{j}")
                    o = st["o"]
                    pab = ptile[i]
                    vt = wsb[it]["vt"]
                    nc.tensor.matmul(o[:, off:512], vt[:, j, 2 * p, :],
                                     pab[:, off:512],
                                     start=(j == 0), stop=(j == njt - 1))
                    nc.tensor.matmul(o[:, 512 + off:1024], vt[:, j, 2 * p + 1, :],
                                     pab[:, 512:1024 - off],
                                     start=(j == 0), stop=(j == njt - 1))
                    st["peT"] += 2 * (512 - off) * PE_C
                    ptile[i] = None
                    if j == njt - 1:
                        norm_chain(it, G, p)

                YRESERVE = 24

                def drain_norms():
                    # zero-cost deferred units (normalization multiplies)
                    while yq and yq[0][2] == 0.0 and st["peT"] >= yq[0][0]:
                        _, fn, _ = yq.popleft()
                        fn()

                def pop_y(force=False):
                    if not yq:
                        return False
                    ready, fn, c = yq[0]
                    if force or (st["peT"] >= ready
                                 and (len(yq) > YRESERVE
                                      or st["fi"] >= len(filler))):
                        yq.popleft()
                        fn()
                        st["peT"] += c
                        return True
                    return False

                si = 0
                ai = 0
                while ai < N:
                    drain_norms()
                    if si <= ai:
                        emit_scores(si)
                        si += 1
                        continue
                    if st["peT"] >= expEnd[ai] + SEM:
                        emit_attn(ai)
                        ai += 1
                        continue
                    # PE needs other work while ACT runs; keep the score
                    # cursor within the attention cursor's iteration
                    ahead_ok = si < N and si - ai < 2
                    if (ahead_ok and scores_dep(si) <= st["fi"]
                            and (si < 2 or st["peT"] >= expEnd[si - 2])):
                        emit_scores(si)
                        si += 1
                        continue
                    if pop_y():
                        continue
                    if st["fi"] < len(filler):
                        emit_filler_one()
                        continue
                    if ahead_ok:
                        emit_scores(si)
                        si += 1
                        continue
                    if pop_y(force=True):
                        continue
                    emit_attn(ai)   # unavoidable stall
                    ai += 1
                # flush remaining work (y of the last groups)
                need(len(filler))
                while pop_y(force=True):
                    pass
                qkvp.release()
                xqp.release()
                wpool.release()

    nc.compile()
    return nc


_RUNNER = None
_INTERNALS = None


def _make_pjrt_runner(nc):
    """Wrap a compiled Bass program as an 8-core PJRT callable."""
    import jax
    import numpy as _np
    from jax.sharding import Mesh, PartitionSpec
    from jax.experimental.shard_map import shard_map
    from concourse import mybir
    from concourse.bass2jax import (_bass_exec_p, install_neuronx_cc_hook,
                                    partition_id_tensor)

    install_neuronx_cc_hook()

    partition_name = (nc.partition_id_tensor.name
                      if nc.partition_id_tensor else None)
    in_names, out_names, out_avals = [], [], []
    for alloc in nc.m.functions[0].allocations:
        if not isinstance(alloc, mybir.MemoryLocationSet):
            continue
        if not alloc.memorylocations:
            continue
        name = alloc.memorylocations[0].name
        if alloc.kind == "ExternalInput":
            if name != partition_name:
                in_names.append(name)
        elif alloc.kind == "ExternalOutput":
            out_names.append(name)
            out_avals.append(jax.core.ShapedArray(
                tuple(alloc.tensor_shape), mybir.dt.np(alloc.dtype)))
    n_params = len(in_names)
    n_outs = len(out_names)
    zero_shapes = [(a.shape, a.dtype) for a in out_avals]
    all_in_names = in_names + out_names
    if partition_name is not None:
        all_in_names = all_in_names + [partition_name]

    def _body(*args):
        operands = list(args)
        if partition_name is not None:
            operands.append(partition_id_tensor())
        outs = _bass_exec_p.bind(
            *operands,
            out_avals=tuple(out_avals),
            in_names=tuple(all_in_names),
            out_names=tuple(out_names),
            lowering_input_output_aliases=(),
            sim_require_finite=True,
            sim_require_nnan=True,
            nc=nc,
        )
        return tuple(outs)

    devices = jax.devices()[:NCORES]
    mesh = Mesh(np.asarray(devices), ("core",))
    sharded = jax.jit(
        shard_map(_body, mesh=mesh,
                  in_specs=(PartitionSpec("core"),) * (n_params + n_outs),
                  out_specs=(PartitionSpec("core"),) * n_outs,
                  check_rep=False),
        keep_unused=True,
    )

    def run(in_maps):
        concat_in = [
            _np.concatenate([_np.asarray(in_maps[c][n]) for c in range(NCORES)],
                            axis=0)
            for n in in_names
        ]
        concat_zeros = [
            _np.zeros((NCORES * s[0], *s[1:]), d) for (s, d) in zero_shapes
        ]
        out_arrs = sharded(*concat_in, *concat_zeros)
        return [
            {
                n: _np.asarray(out_arrs[i]).reshape(NCORES, *out_avals[i].shape)[c]
                for i, n in enumerate(out_names)
            }
            for c in range(NCORES)
        ]

    internals = dict(nc=nc, body=_body, mesh=mesh, in_names=in_names,
                     out_names=out_names, zero_shapes=zero_shapes,
                     n_params=n_params)
    return run, in_names, internals


def _get_runner():
    """Build the Bass program once and return a cached 8-core PJRT callable."""
    global _RUNNER, _INTERNALS
    if _RUNNER is not None:
        return _RUNNER
    run, in_names, internals = _make_pjrt_runner(_build_program())
    _INTERNALS = internals
    _RUNNER = (run, in_names)
    return _RUNNER


def _make_in_maps(x, wq, wk, wv, wo):
    import ml_dtypes
    BF = ml_dtypes.bfloat16
    x = np.asarray(x, np.float32)
    wq_s = np.asarray(wq, np.float32) * (1.0 / np.sqrt(HD))  # fold score scale
    wk = np.asarray(wk, np.float32)
    wv = np.asarray(wv, np.float32)
    wo = np.asarray(wo, np.float32)

    xt_b = [np.ascontiguousarray(x[b].T).astype(BF) for b in range(B)]
    in_maps = []
    for c in range(NCORES):
        b, g = c // GROUPS, c % GROUPS
        sl = slice(DH * g, DH * (g + 1))
        in_maps.append({
            "xt": xt_b[b],
            "wqt": np.ascontiguousarray(wq_s[sl, :].T).astype(BF),
            "wkt": np.ascontiguousarray(wk[sl, :].T).astype(BF),
            "wvt": np.ascontiguousarray(wv[sl, :].T).astype(BF),
            "wot": np.ascontiguousarray(wo[:, sl].T).astype(BF),
        })
    return in_maps


def kernel(x, wq, wk, wv, wo):
    run, _ = _get_runner()
    results = run(_make_in_maps(x, wq, wk, wv, wo))
    y = np.zeros((B, T, DIM), np.float32)
    for c in range(NCORES):
        y[c // GROUPS] += results[c]["y"]
    return y
